# revision 24
# baseline (speedup 1.0000x reference)
"""Trainium2 Bass kernel for the vq_codebook CCE loss.

Live dataflow of the reference:
    t   = (1/(B*F)) * sum_b min_p ||outputs[b] - clusters[tc_b, p]||^2
    out = ALPHA*t + BETA*(1 - t)
Only the TARGET class's prototype distances feed the loss (the wrong-class
branch of the reference is dead code), so per batch row only 32 of the
6400 prototype distances are live.

Strategy (8 NeuronCores, SPMD):
  - Host sorts rows by target class (stable) and splits the sorted batch
    into 16 tiles of 128 rows.  Each tile's rows span a small contiguous
    class range (<=16 classes for random data), so a single 512-column
    PSUM bank holds every prototype column any of its rows needs.
  - Each core takes 2 tiles.  Per tile: 3 fp8 DoubleRow matmuls (256
    contraction rows each) compute -2*x.c for the gathered columns, plus
    one DoubleRow rank-2 matmul adds ||c||^2 (split 16*h + r, both fp8,
    abs err <= 2).  DVE takes a windowed min over each class's 32
    prototypes, then a fused mask-select+sum picks each row's own class.
  - ||x||^2 comes from a single Scalar-engine Square pass with accum_out
    over the core's fp8 x slice.
  - Host combines: t = (sum x2 + sum selected_min)/(B*F).

fp8 e4m3 quantization moves t by ~0.03% (validated off-device vs f64).
"""

import numpy as np
import ml_dtypes  # noqa: F401  (np dtype registry for bf16/fp8)
from contextlib import ExitStack

import concourse.tile as tile
from concourse import bacc, mybir
from concourse.tile import add_dep_helper
from concourse.bass_utils import run_bass_kernel_spmd

ALPHA = 5.0
BETA = 5.0

B, F, C, P = 2048, 768, 200, 32
NCORES = 8
NT = B // 128            # 16 row tiles of 128 sorted rows
TPC = NT // NCORES       # 2 tiles per core
K3 = F // 256            # 3 DoubleRow contraction chunks
RPC = 128 * TPC          # 256 rows per core

F32 = mybir.dt.float32
BF16 = mybir.dt.bfloat16
KDT = mybir.dt.float8e4
AX = mybir.AxisListType
OP = mybir.AluOpType

_prog_cache = {}

import os
V_C2 = os.environ.get("KV_C2", "dr")      # dr | bf16
V_X2 = os.environ.get("KV_X2", "act")     # act | off
V_MM = os.environ.get("KV_MM", "dr")      # dr | plain
# NOTE: tensor_tensor_reduce crashes the exec unit on this HW (bisected);
# keep the split gpsimd-mult + vector-reduce form.
V_SEL = os.environ.get("KV_SEL", "split")   # ttr | split
V_DMA = os.environ.get("KV_DMA", "3")       # cg pieces: 1 | 3 | 6


def _build_program(nb):
    """nb = PSUM banks per tile (1 unless some tile spans >16 classes)."""
    key = ("nc", nb, V_C2, V_X2, V_MM, V_SEL, V_DMA)
    if key in _prog_cache:
        return _prog_cache[key]

    ncol = 512 * nb          # prototype columns per tile
    nw = ncol // 32          # class windows per tile

    nc = bacc.Bacc(
        "TRN2", target_bir_lowering=False, debug=False, num_devices=NCORES,
        enable_asserts=False, enable_partition_id=False,
    )

    xa = nc.dram_tensor("xa", [128, K3 * 2 * RPC], KDT, kind="ExternalInput").ap()
    cg = nc.dram_tensor("cg", [128, K3 * TPC * 2 * ncol], KDT, kind="ExternalInput").ap()
    # per tile: h row (ncol), r row (ncol); then lhsT consts (16.0)*128, (1.0)*128
    mb = nc.dram_tensor("mb", [1, TPC * 2 * ncol + 256], KDT, kind="ExternalInput").ap()
    m2 = (
        nc.dram_tensor("m2", [1, TPC * ncol + 128], BF16, kind="ExternalInput").ap()
        if V_C2 == "bf16" else None
    )
    mk = nc.dram_tensor("mk", [128, TPC * nw], F32, kind="ExternalInput").ap()
    out = nc.dram_tensor("out", [128, 3], F32, kind="ExternalOutput").ap()

    DR = mybir.MatmulPerfMode.DoubleRow

    with tile.TileContext(nc) as tc, ExitStack() as ctx:
        const = ctx.enter_context(tc.tile_pool(name="const", bufs=1))
        psum = ctx.enter_context(tc.tile_pool(name="psum", bufs=2 * nb, space="PSUM"))

        xa_sb = const.tile([128, K3 * 2 * RPC], KDT, name="xa_sb", tag="xa")
        cg_sb = const.tile([128, K3 * TPC * 2 * ncol], KDT, name="cg_sb", tag="cg")
        mb_sb = const.tile([1, TPC * 2 * ncol + 256], KDT, name="mb_sb", tag="mb")
        m2_sb = (
            const.tile([1, TPC * ncol + 128], BF16, name="m2_sb", tag="m2")
            if V_C2 == "bf16" else None
        )
        mk_sb = const.tile([128, TPC * nw], F32, name="mk_sb", tag="mk")
        mwin = const.tile([128, TPC * nw], F32, name="mwin", tag="mw")
        junk = const.tile([128, TPC * nw], F32, name="junk", tag="jk")
        sq = const.tile([128, K3 * 2 * RPC], BF16, name="sq", tag="sq")
        res = const.tile([128, 3], F32, name="res", tag="res")

        xa_v = xa_sb[:].rearrange("p (k s r) -> p k s r", k=K3, s=2)
        cg_v = cg_sb[:].rearrange("p (k t s j) -> p k t s j", k=K3, t=TPC, s=2)
        mb_v = mb_sb[:, 0 : TPC * 2 * ncol].rearrange(
            "p (t s j) -> p t s j", t=TPC, s=2
        )
        ones2 = mb_sb[:, TPC * 2 * ncol :].rearrange("p (s r) -> p s r", s=2)

        # --- DMAs: no dep chains (each chained link pays ~2us completion
        # latency).  cg streams on the sync HWDGE ring in chunk order; xa
        # and the small tensors ride the scalar HWDGE ring in parallel. ---
        if V_DMA == "1":
            nc.sync.dma_start(cg_sb[:], cg)
        elif V_DMA == "3":
            cg_f = cg_sb[:].rearrange("p (k x) -> p k x", k=K3)
            cg_d = cg.rearrange("p (k x) -> p k x", k=K3)
            for k in range(K3):
                nc.sync.dma_start(cg_f[:, k, :], cg_d[:, k, :])
        else:
            cg_f = cg_sb[:].rearrange("p (k t x) -> p k t x", k=K3, t=TPC)
            cg_d = cg.rearrange("p (k t x) -> p k t x", k=K3, t=TPC)
        nc.scalar.dma_start(xa_sb[:], xa)
        if V_DMA == "6":
            for k in range(K3):
                nc.sync.dma_start(cg_f[:, k, 0, :], cg_d[:, k, 0, :])
                nc.scalar.dma_start(cg_f[:, k, 1, :], cg_d[:, k, 1, :])
        nc.scalar.dma_start(mb_sb[:], mb)
        if m2_sb is not None:
            nc.scalar.dma_start(m2_sb[:], m2)
        nc.scalar.dma_start(mk_sb[:], mk)

        # --- Sum x^2 on the Scalar engine in the DMA/PE shadow ---
        if V_X2 == "act":
            nc.scalar.activation(
                out=sq[:], in_=xa_sb[:],
                func=mybir.ActivationFunctionType.Square,
                accum_out=res[:, 2:3],
            )
        else:
            nc.gpsimd.memset(res[:, 2:3], 0.0)

        # --- PE: per tile, 3 DoubleRow chunks + rank-2 c2 add.
        # The c2 rank-2 matmul goes BEFORE the last chunk (it only needs the
        # tiny mb tensor), so the windowed min can fire the moment the final
        # cg chunk's matmul retires. ---
        pss = [psum.tile([128, ncol], F32, name="ps", tag="ps") for _ in range(TPC)]

        def mm_chunk(t, k, b, start, stop):
            if V_MM == "dr":
                nc.tensor.matmul(
                    pss[t][:, b * 512 : (b + 1) * 512],
                    lhsT=xa_v[:, k, :, t * 128 : (t + 1) * 128],
                    rhs=cg_v[:, k, t, :, b * 512 : (b + 1) * 512],
                    perf_mode=DR,
                    start=start,
                    stop=stop,
                )
            else:
                for s in range(2):
                    nc.tensor.matmul(
                        pss[t][:, b * 512 : (b + 1) * 512],
                        lhsT=xa_v[:, k, s, t * 128 : (t + 1) * 128],
                        rhs=cg_v[:, k, t, s, b * 512 : (b + 1) * 512],
                        start=(start and s == 0),
                        stop=(stop and s == 1),
                    )

        def mm_c2(t, b):
            if V_C2 == "dr":
                nc.tensor.matmul(
                    pss[t][:, b * 512 : (b + 1) * 512],
                    lhsT=ones2,
                    rhs=mb_v[:, t, :, b * 512 : (b + 1) * 512],
                    perf_mode=DR,
                    start=False,
                    stop=False,
                )
            else:
                nc.tensor.matmul(
                    pss[t][:, b * 512 : (b + 1) * 512],
                    lhsT=m2_sb[:, TPC * ncol : TPC * ncol + 128],
                    rhs=m2_sb[:, t * ncol + b * 512 : t * ncol + (b + 1) * 512],
                    start=False,
                    stop=False,
                )

        for k in range(K3 - 1):
            for t in range(TPC):
                for b in range(nb):
                    mm_chunk(t, k, b, start=(k == 0), stop=False)
        for t in range(TPC):
            for b in range(nb):
                mm_c2(t, b)
        for t in range(TPC):
            for b in range(nb):
                mm_chunk(t, K3 - 1, b, start=False, stop=True)

        # --- DVE: windowed min over 32 prototypes, then mask-select+sum ---
        for t in range(TPC):
            nc.vector.tensor_reduce(
                out=mwin[:, t * nw : (t + 1) * nw],
                in_=pss[t][:].rearrange("p (w x) -> p w x", x=P),
                axis=AX.X,
                op=OP.min,
            )
            if V_SEL == "ttr":
                nc.vector.tensor_tensor_reduce(
                    out=junk[:, t * nw : (t + 1) * nw],
                    in0=mwin[:, t * nw : (t + 1) * nw],
                    in1=mk_sb[:, t * nw : (t + 1) * nw],
                    scale=1.0,
                    scalar=0.0,
                    op0=OP.mult,
                    op1=OP.add,
                    accum_out=res[:, t : t + 1],
                )
            else:
                nc.gpsimd.tensor_tensor(
                    out=junk[:, t * nw : (t + 1) * nw],
                    in0=mwin[:, t * nw : (t + 1) * nw],
                    in1=mk_sb[:, t * nw : (t + 1) * nw],
                    op=OP.mult,
                )
                nc.vector.tensor_reduce(
                    out=res[:, t : t + 1],
                    in_=junk[:, t * nw : (t + 1) * nw],
                    axis=AX.X,
                    op=OP.add,
                )

        nc.sync.dma_start(out, res[:])

    nc.compile()
    _prog_cache[key] = nc
    return nc


def _prep_inputs(outputs, clusters, target_classes):
    outputs = np.ascontiguousarray(np.asarray(outputs, dtype=np.float32))
    clusters = np.ascontiguousarray(np.asarray(clusters, dtype=np.float32))
    tc_np = np.asarray(target_classes).astype(np.int64)

    np_k = mybir.dt.np(KDT)
    np_b = mybir.dt.np(BF16)

    order = np.argsort(tc_np, kind="stable")
    xs = outputs[order]
    tcs = tc_np[order]

    los = np.empty(NT, np.int64)
    spans = np.empty(NT, np.int64)
    for t in range(NT):
        seg = tcs[t * 128 : (t + 1) * 128]
        los[t] = seg.min()
        spans[t] = seg.max() - seg.min() + 1
    nb = max(1, int(-(-int(spans.max()) // 16)))
    ncol = 512 * nb
    nw = ncol // 32

    flat = clusters.reshape(C * P, F)
    c2 = (flat.astype(np.float64) ** 2).sum(axis=1).astype(np.float32)

    # -2x in fp8, laid out (p, k, s, r): feature = k*256 + s*128 + p
    a8 = np.clip(-2.0 * xs, -240, 240).astype(np_k)  # [B, F]

    in_maps = []
    for ci in range(NCORES):
        rows = slice(ci * RPC, (ci + 1) * RPC)
        xa_i = np.ascontiguousarray(
            a8[rows].T.reshape(K3, 2, 128, RPC).transpose(2, 0, 1, 3)
            .reshape(128, K3 * 2 * RPC)
        )

        cg_i = np.zeros((128, K3, TPC, 2, ncol), np_k)
        mb_i = np.zeros((1, TPC * 2 * ncol + 256), np_k)
        m2_i = np.zeros((1, TPC * ncol + 128), np_b)
        mk_i = np.zeros((128, TPC * nw), np.float32)
        for tt in range(TPC):
            t = ci * TPC + tt
            lo = int(los[t])
            hi = min(lo + nw, C)
            npro = (hi - lo) * P
            G = flat[lo * P : hi * P]                       # [npro, F]
            g8 = np.clip(G, -240, 240).astype(np_k)
            # (F, npro) -> (k, s, p, npro) -> (p, k, s, npro)
            cg_i[:, :, tt, :, :npro] = (
                g8.T.reshape(K3, 2, 128, npro).transpose(2, 0, 1, 3)
            )
            c2t = np.zeros(ncol, np.float32)
            c2t[:npro] = c2[lo * P : hi * P]
            h8 = np.clip(c2t / 16.0, -240, 240).astype(np_k)
            r8 = np.clip(c2t - 16.0 * h8.astype(np.float32), -240, 240).astype(np_k)
            mb_i[0, tt * 2 * ncol : tt * 2 * ncol + ncol] = h8
            mb_i[0, tt * 2 * ncol + ncol : (tt + 1) * 2 * ncol] = r8
            m2_i[0, tt * ncol : (tt + 1) * ncol] = c2t.astype(np_b)
            w = tcs[t * 128 : (t + 1) * 128] - lo           # [128] window idx
            mk_i[np.arange(128), tt * nw + w] = 1.0
        mb_i[0, TPC * 2 * ncol : TPC * 2 * ncol + 128] = np.float32(16.0).astype(np_k)
        mb_i[0, TPC * 2 * ncol + 128 :] = np.float32(1.0).astype(np_k)
        m2_i[0, TPC * ncol :] = np.float32(1.0).astype(np_b)

        im = {
            "xa": xa_i,
            "cg": np.ascontiguousarray(cg_i.reshape(128, -1)),
            "mb": mb_i,
            "mk": mk_i,
        }
        if V_C2 == "bf16":
            im["m2"] = m2_i
        in_maps.append(im)
    return nb, in_maps


def _finish(results):
    s = 0.0
    for r in results:
        o = r["out"].astype(np.float64)
        s += o[:, 0].sum() + o[:, 1].sum() + o[:, 2].sum() / 4.0
    t = np.float32(s / (B * F))
    ans = np.float32(ALPHA) * t + np.float32(BETA) * (np.float32(1.0) - t)
    return np.asarray(ans, dtype=np.float32)


def kernel(outputs, clusters, target_classes, _run_kwargs=None):
    nb, in_maps = _prep_inputs(outputs, clusters, target_classes)
    nc = _build_program(nb)
    kw = _run_kwargs or {}
    res = run_bass_kernel_spmd(nc, in_maps, list(range(NCORES)), **kw)
    ans = _finish(res.results)
    if _run_kwargs is not None:
        kernel.last_result = res
    return ans


if __name__ == "__main__":
    rng = np.random.default_rng(0)
    o = rng.standard_normal((B, F), dtype=np.float32)
    cl = rng.standard_normal((C, P, F), dtype=np.float32)
    t = rng.integers(0, C, size=(B,)).astype(np.int32)
    print(kernel(o, cl, t))


# revision 26
# speedup vs baseline: 1.1497x; 1.1497x over previous
"""Trainium2 Bass kernel for the vq_codebook CCE loss.

Live dataflow of the reference:
    t   = (1/(B*F)) * sum_b min_p ||outputs[b] - clusters[tc_b, p]||^2
    out = ALPHA*t + BETA*(1 - t)
Only the TARGET class's prototype distances feed the loss (the wrong-class
branch of the reference is dead code), so per batch row only 32 of the
6400 prototype distances are live.

Strategy (8 NeuronCores, SPMD):
  - Host sorts rows by target class (stable) and splits the sorted batch
    into 16 tiles of 128 rows.  Each tile's rows span a small contiguous
    class range (<=16 classes for random data), so a single 512-column
    PSUM bank holds every prototype column any of its rows needs.
  - Each core takes 2 tiles.  Per tile: 3 fp8 DoubleRow matmuls (256
    contraction rows each) compute -2*x.c for the gathered columns, then
    one rank-20 DoubleRow matmul adds ||c||^2 (split 16*h + r, fp8,
    abs err <= 2) AND a +224*224 shift on every column outside the row's
    own class window (the one-hot select mask is rank-16, so it rides the
    same matmul; the shift cancels exactly on the row's own window).
  - A single full-row DVE min per tile then yields each row's selected
    nearest-prototype distance directly - no mask/select stage.
  - ||x||^2 comes from one Scalar-engine Square pass with accum_out over
    the core's fp8 x slice (a = -2x, so sum a^2 = 4 sum x^2).
  - A final f32 ones-matmul reduces [128,3] partials across partitions to
    [1,3] so the output DMA is a single descriptor.
  - Host combines: t = (sum x2 + sum selected_min)/(B*F).

fp8 e4m3 quantization moves t by ~0.03% (validated off-device vs f64).
NOTE: tensor_tensor_reduce crashes the exec unit on this HW (bisected);
do not reintroduce it.
"""

import os
import numpy as np
import ml_dtypes  # noqa: F401  (np dtype registry for bf16/fp8)
from contextlib import ExitStack

import concourse.tile as tile
from concourse import bacc, mybir
from concourse.bass_utils import run_bass_kernel_spmd

ALPHA = 5.0
BETA = 5.0

B, F, C, P = 2048, 768, 200, 32
NCORES = 8
NT = B // 128            # 16 row tiles of 128 sorted rows
TPC = NT // NCORES       # 2 tiles per core
K3 = F // 256            # 3 DoubleRow contraction chunks
RPC = 128 * TPC          # 256 rows per core
BIG = 224.0              # BIG*BIG = 50176 shift for non-selected columns

F32 = mybir.dt.float32
BF16 = mybir.dt.bfloat16
KDT = mybir.dt.float8e4
AX = mybir.AxisListType
OP = mybir.AluOpType

V_DMA = os.environ.get("KV_DMA", "3")  # cg stream pieces: 1 | 3

_prog_cache = {}


def _build_program(nb):
    """nb = PSUM banks per tile (1 unless some tile spans >16 classes)."""
    key = ("nc", nb, V_DMA)
    if key in _prog_cache:
        return _prog_cache[key]

    ncol = 512 * nb          # prototype columns per tile
    mbr = TPC * 2 * ncol     # rhs part of mb
    mbl = TPC * nb * 256     # lhsT part of mb

    nc = bacc.Bacc(
        "TRN2", target_bir_lowering=False, debug=False, num_devices=NCORES,
        enable_asserts=False, enable_partition_id=False,
    )

    xa = nc.dram_tensor("xa", [128, K3 * 2 * RPC], KDT, kind="ExternalInput").ap()
    cg = nc.dram_tensor("cg", [128, K3 * TPC * 2 * ncol], KDT, kind="ExternalInput").ap()
    mb = nc.dram_tensor("mb", [10, mbr + mbl], KDT, kind="ExternalInput").ap()
    out = nc.dram_tensor("out", [1, 3], F32, kind="ExternalOutput").ap()

    DR = mybir.MatmulPerfMode.DoubleRow

    with tile.TileContext(nc) as tc, ExitStack() as ctx:
        const = ctx.enter_context(tc.tile_pool(name="const", bufs=1))
        psum = ctx.enter_context(tc.tile_pool(name="psum", bufs=2 * nb, space="PSUM"))
        psco = ctx.enter_context(tc.tile_pool(name="psco", bufs=1, space="PSUM"))

        xa_sb = const.tile([128, K3 * 2 * RPC], KDT, name="xa_sb", tag="xa")
        cg_sb = const.tile([128, K3 * TPC * 2 * ncol], KDT, name="cg_sb", tag="cg")
        mb_sb = const.tile([10, mbr + mbl], KDT, name="mb_sb", tag="mb")
        sq = const.tile([128, K3 * 2 * RPC], BF16, name="sq", tag="sq")
        res = const.tile([128, 3], F32, name="res", tag="res")
        ones = const.tile([128, 1], F32, name="ones", tag="on")
        outs = const.tile([1, 3], F32, name="outs", tag="os")

        xa_v = xa_sb[:].rearrange("p (k s r) -> p k s r", k=K3, s=2)
        cg_v = cg_sb[:].rearrange("p (k t s j) -> p k t s j", k=K3, t=TPC, s=2)
        mbr_v = mb_sb[:, 0:mbr].rearrange("p (t s j) -> p t s j", t=TPC, s=2)
        mbl_v = mb_sb[:, mbr:].rearrange("p (t b s r) -> p t b s r", t=TPC, b=nb, s=2)

        # --- DMAs: no dep chains; cg streams on the sync HWDGE ring in
        # chunk order, xa + mb ride the scalar HWDGE ring in parallel. ---
        if V_DMA == "1":
            nc.sync.dma_start(cg_sb[:], cg)
        else:
            cg_f = cg_sb[:].rearrange("p (k x) -> p k x", k=K3)
            cg_d = cg.rearrange("p (k x) -> p k x", k=K3)
            for k in range(K3):
                nc.sync.dma_start(cg_f[:, k, :], cg_d[:, k, :])
        nc.scalar.dma_start(xa_sb[:], xa)
        nc.scalar.dma_start(mb_sb[:], mb)

        nc.gpsimd.memset(ones[:], 1.0)

        # --- sum x^2 on the Scalar engine in the DMA/PE shadow ---
        nc.scalar.activation(
            out=sq[:], in_=xa_sb[:],
            func=mybir.ActivationFunctionType.Square,
            accum_out=res[:, 2:3],
        )

        # --- PE: per tile, 2 DoubleRow chunks, the rank-20 c2+select
        # matmul (needs only mb), then the last chunk with stop=True so
        # the DVE min fires the moment it retires. ---
        pss = [psum.tile([128, ncol], F32, name="ps", tag="ps") for _ in range(TPC)]

        def mm_chunk(t, k, start, stop):
            for b in range(nb):
                nc.tensor.matmul(
                    pss[t][:, b * 512 : (b + 1) * 512],
                    lhsT=xa_v[:, k, :, t * 128 : (t + 1) * 128],
                    rhs=cg_v[:, k, t, :, b * 512 : (b + 1) * 512],
                    perf_mode=DR,
                    start=start,
                    stop=stop,
                )

        for k in range(K3 - 1):
            for t in range(TPC):
                mm_chunk(t, k, start=(k == 0), stop=False)
        for t in range(TPC):
            for b in range(nb):
                nc.tensor.matmul(
                    pss[t][:, b * 512 : (b + 1) * 512],
                    lhsT=mbl_v[:, t, b, :, :],
                    rhs=mbr_v[:, t, :, b * 512 : (b + 1) * 512],
                    perf_mode=DR,
                    start=False,
                    stop=False,
                )
        for t in range(TPC):
            mm_chunk(t, K3 - 1, start=False, stop=True)

        # --- DVE: one full-row min per tile = the selected distance ---
        for t in range(TPC):
            nc.vector.tensor_reduce(
                out=res[:, t : t + 1],
                in_=pss[t][:],
                axis=AX.X,
                op=OP.min,
            )

        # --- cross-partition reduce on the PE, then a 1-descriptor DMA ---
        pco = psco.tile([1, 3], F32, name="pco", tag="pco")
        nc.tensor.matmul(pco[:], lhsT=ones[:], rhs=res[:], start=True, stop=True)
        nc.scalar.copy(out=outs[:], in_=pco[:])
        nc.sync.dma_start(out, outs[:])

    nc.compile()
    _prog_cache[key] = nc
    return nc


def _prep_inputs(outputs, clusters, target_classes):
    outputs = np.ascontiguousarray(np.asarray(outputs, dtype=np.float32))
    clusters = np.ascontiguousarray(np.asarray(clusters, dtype=np.float32))
    tc_np = np.asarray(target_classes).astype(np.int64)

    np_k = mybir.dt.np(KDT)

    order = np.argsort(tc_np, kind="stable")
    xs = outputs[order]
    tcs = tc_np[order]

    los = np.empty(NT, np.int64)
    spans = np.empty(NT, np.int64)
    for t in range(NT):
        seg = tcs[t * 128 : (t + 1) * 128]
        los[t] = seg.min()
        spans[t] = seg.max() - seg.min() + 1
    nb = max(1, int(-(-int(spans.max()) // 16)))
    ncol = 512 * nb
    nw = ncol // 32
    mbr = TPC * 2 * ncol
    mbl = TPC * nb * 256

    flat = clusters.reshape(C * P, F)
    c2 = (flat.astype(np.float64) ** 2).sum(axis=1).astype(np.float32)

    # -2x in fp8, laid out (p, k, s, r): feature = k*256 + s*128 + p
    a8 = np.clip(-2.0 * xs, -240, 240).astype(np_k)  # [B, F]

    big8 = np.float32(BIG).astype(np_k)

    in_maps = []
    for ci in range(NCORES):
        rows = slice(ci * RPC, (ci + 1) * RPC)
        xa_i = np.ascontiguousarray(
            a8[rows].T.reshape(K3, 2, 128, RPC).transpose(2, 0, 1, 3)
            .reshape(128, K3 * 2 * RPC)
        )

        cg_i = np.zeros((128, K3, TPC, 2, ncol), np_k)
        mb_i = np.zeros((10, mbr + mbl), np_k)
        mbr_v = mb_i[:, 0:mbr].reshape(10, TPC, 2, ncol)
        mbl_v = mb_i[:, mbr:].reshape(10, TPC, nb, 2, 128)
        for tt in range(TPC):
            t = ci * TPC + tt
            lo = int(los[t])
            hi = min(lo + nw, C)
            npro = (hi - lo) * P
            G = flat[lo * P : hi * P]                       # [npro, F]
            g8 = np.clip(G, -240, 240).astype(np_k)
            # (F, npro) -> (k, s, p, npro) -> (p, k, s, npro)
            cg_i[:, :, tt, :, :npro] = (
                g8.T.reshape(K3, 2, 128, npro).transpose(2, 0, 1, 3)
            )
            c2t = np.zeros(ncol, np.float32)
            c2t[:npro] = c2[lo * P : hi * P]
            h8 = np.clip(c2t / 16.0, -240, 240).astype(np_k)
            r8 = np.clip(c2t - 16.0 * h8.astype(np.float32), -240, 240).astype(np_k)
            # rhs components (partition kk, slot s) = comp 2*kk+s:
            #   0: h, 1: r, 2: +BIG const, 3+w: -BIG on window w's columns
            mbr_v[0, tt, 0, :] = h8
            mbr_v[0, tt, 1, :] = r8
            mbr_v[1, tt, 0, :] = big8
            wincol = np.repeat(np.arange(nw), P)            # window of each col
            for w in range(nw):
                comp = 3 + (w % 16)                         # bank-local component
                mbr_v[comp // 2, tt, comp % 2, wincol == w] = -big8
            # lhsT components: 0: 16, 1: 1, 2: BIG, 3+w: BIG iff row's
            # window == w (per bank: component 3+wl maps window b*16+wl)
            w_r = (tcs[t * 128 : (t + 1) * 128] - lo).astype(np.int64)
            mbl_v[0, tt, :, 0, :] = np.float32(16.0).astype(np_k)
            mbl_v[0, tt, :, 1, :] = np.float32(1.0).astype(np_k)
            mbl_v[1, tt, :, 0, :] = big8
            for bk in range(nb):
                for wl in range(16):
                    comp = 3 + wl
                    sel = w_r == bk * 16 + wl
                    mbl_v[comp // 2, tt, bk, comp % 2, sel] = big8

        in_maps.append(
            {
                "xa": xa_i,
                "cg": np.ascontiguousarray(cg_i.reshape(128, -1)),
                "mb": np.ascontiguousarray(mb_i),
            }
        )
    return nb, in_maps


def _finish(results):
    s = 0.0
    for r in results:
        o = r["out"].astype(np.float64)
        s += o[0, 0] + o[0, 1] + o[0, 2] / 4.0
    t = np.float32(s / (B * F))
    ans = np.float32(ALPHA) * t + np.float32(BETA) * (np.float32(1.0) - t)
    return np.asarray(ans, dtype=np.float32)


def kernel(outputs, clusters, target_classes, _run_kwargs=None):
    nb, in_maps = _prep_inputs(outputs, clusters, target_classes)
    nc = _build_program(nb)
    kw = _run_kwargs or {}
    res = run_bass_kernel_spmd(nc, in_maps, list(range(NCORES)), **kw)
    ans = _finish(res.results)
    if _run_kwargs is not None:
        kernel.last_result = res
    return ans


if __name__ == "__main__":
    rng = np.random.default_rng(0)
    o = rng.standard_normal((B, F), dtype=np.float32)
    cl = rng.standard_normal((C, P, F), dtype=np.float32)
    t = rng.integers(0, C, size=(B,)).astype(np.int32)
    print(kernel(o, cl, t))
